# revision 18
# baseline (speedup 1.0000x reference)
"""Trainium2 Bass kernel for DigitConvolutionalModel (8-core data parallel).

Computation: x(B,784) -> 3x3 valid conv on 28x28 -> flatten(676)
             -> FC(100)+ReLU -> FC(10), B = 65536.

Algebraic restructure (host side, exact): the conv is linear, so conv and
fc1 fold into one 784->100 matrix W1eff (accumulated in float64). The
device kernel is then two matmul layers per 512-sample tile:
  h = relu(x @ W1eff + b1);  y = h @ fc2_w.T + b2.

Numerics: x is streamed as fp8 e3m4 (scaled x2 on the host; the 1/2 is
folded into W1eff which stays fp16). This halves HBM traffic vs fp16 --
the kernel's roofline -- at a measured end-to-end scale-relative absmax
error of ~1.2e-2 vs the fp32 reference (gate 2e-2). The matmul runs in
normal (non-DoubleRow) mode: moving operand e3m4, stationary fp16.

Per-core layout (B_shard=8192 = 16 tiles x 512, batches of 4/4/4/2/2):
  x feature-major: 6 chunks of 128 features on SBUF partitions, 2-tile
  786KB DMAs alternating between the SP and ACT HWDGE rings (each ring
  holds ~2 DMAs of 128 descriptors -- single-ring issue stalls starve
  the PE). The 16-feature remainder sits at 32-aligned partition groups
  (g = t%4) so a batch's remainder matmuls land in distinct PE row
  groups and overlap. Layer-2 matmuls are col-tiled at out partitions
  32q so a batch's matmuls overlap in distinct PE column groups. W1 is
  padded to 128 output columns (FWL). relu alternates DVE/ACT by tile;
  PSUM->SBUF copies split by column-group parity into two staging tiles
  (o_ev on DVE, o_od on ACT) so the two engines never write the same
  tile (whole-tile WAW tracking would serialize them). Outputs leave as
  16 small per-tile DMAs: SBUF->HBM stores only engage 2 of 16 SDMA
  engines (~41 GB/s), so one big store would add ~5us of tail. The last
  two batches are 2 tiles so the final relu->fc2->copy->store chain is
  short. Full-width fp8 warmup matmuls advance the HAM clock ramp while
  x0 streams in.
"""

import numpy as np
import ml_dtypes

import concourse.bass as bass
import concourse.mybir as mybir
import concourse.tile as tile
from concourse.bass_utils import run_bass_kernel_spmd
from concourse.vector_clock import ScopedClock

N_CORES = 8
B_TOTAL = 65536
B_SHARD = B_TOTAL // N_CORES  # 8192
BT = 512  # batch tile (one PSUM bank of fp32)
N_TILES = B_SHARD // BT  # 16
FC = 6  # full 128-partition feature chunks (6*128 = 768)
F_REM = 784 - FC * 128  # 16 remainder features
H1 = 100
H2 = 10
MP = 128  # W1 output columns padded 100 -> 128 (FWL trigger)
SX = 2.0  # x pre-scale before e3m4 quantization (1/SX folded into W1)
N_WARM = 8  # full-width warmup matmuls for the HAM clock ramp
BATCHES = [(0, 4), (4, 4), (8, 4), (12, 2), (14, 1), (15, 1)]

_f32 = mybir.dt.float32
_f16 = mybir.dt.float16
_f8 = mybir.dt.float8e3

_RELU = mybir.ActivationFunctionType.Relu
_IDENT = mybir.ActivationFunctionType.Identity


class SplitDrainTileContext(tile.TileContext):
    """TileContext whose tail drain carries at most one sync wait.

    The pinned walrus rejects instructions with >2 sync waits
    ("Too many sync wait commands" in setupSyncWait); the stock tail
    drain accumulates one wait per active proc. Emit one drain per
    wait instead — consecutive drains on the sync engine are
    semantically equivalent to one drain carrying all the waits.
    """

    def _drain_and_barrier(self, tick_clock, wait_clock):
        nc = self.nc
        drain_inst = nc.gpsimd.drain()
        wait_clock.add_sem_waits(
            drain_inst.ins, ScopedClock({None: tick_clock.global_clock})
        )
        raw = drain_inst.ins
        si = raw.sync_info
        if si is not None and si.on_wait and len(si.on_wait) > 1:
            waits = list(si.on_wait)
            si.on_wait = waits[:1]
            raw.sync_info = si
            for w in waits[1:]:
                extra = nc.gpsimd.drain()
                extra.ins.sync_info = mybir.SyncInfo(on_wait=[w], on_update=[])
        for eng in (nc.sync, nc.scalar, nc.vector, nc.tensor):
            eng.drain()

        assert self.sems is not None
        popped = nc._tile_sem_poison_stack.pop()
        assert popped is self._sem_poison
        nc.clear_and_free_semaphores(list(self.sems.allocated().values()))


def _split_sync_waits(nc: bass.Bass, limit: int = 1) -> None:
    """Walrus-compat post-pass: hoist excess sync waits onto NoOps."""
    n = 0
    for fn in nc.m.functions:
        for bb in fn.blocks:
            out = []
            changed = False
            for inst in bb.instructions:
                si = inst.sync_info
                if si is not None and si.on_wait and len(si.on_wait) > limit:
                    waits = list(si.on_wait)
                    for i in range(0, len(waits) - limit, limit):
                        nop = mybir.InstNoOp(
                            name=f"swsplit-{n}",
                            ins=[],
                            outs=[],
                            sync_info=mybir.SyncInfo(
                                on_wait=waits[i : i + limit], on_update=[]
                            ),
                        )
                        nop.engine = inst.engine
                        out.append(nop)
                        n += 1
                    si.on_wait = waits[len(waits) - limit :]
                    inst.sync_info = si
                    changed = True
                out.append(inst)
            if changed:
                bb.instructions = out


def _build_nc() -> bass.Bass:
    nc = bass.Bass(monotonic_sem_count=0)
    # x: 6 feature chunks, tile-major then chunk then batch per partition
    xall = nc.dram_tensor("xall", [128, N_TILES * FC * BT], _f8, kind="ExternalInput")
    # remainder features at 32-aligned partition groups (rows 32g+k used)
    xr = nc.dram_tensor("xr", [112, 4 * BT], _f8, kind="ExternalInput")
    # W1: 6 main chunk blocks + remainder block (replicated at 32-offsets)
    w1 = nc.dram_tensor("w1", [128, (FC + 1) * MP], _f16, kind="ExternalInput")
    w2 = nc.dram_tensor("w2", [H1, H2], _f16, kind="ExternalInput")
    b1 = nc.dram_tensor("b1", [H1, 1], _f32, kind="ExternalInput")
    b2 = nc.dram_tensor("b2", [H2, 1], _f32, kind="ExternalInput")
    # y[32q+p, s*BT+n] = out[(4s+q)*BT+n, p]; rows 32q+10..31 are junk
    y = nc.dram_tensor("y", [106, 4 * BT], _f32, kind="ExternalOutput")

    with SplitDrainTileContext(nc) as tc:
        with (
            tc.tile_pool(name="consts", bufs=1) as cpool,
            tc.tile_pool(name="hp", bufs=4) as hpool,
            tc.tile_pool(name="psh", bufs=6, space="PSUM") as psh,
            tc.tile_pool(name="pso", bufs=2, space="PSUM") as pso,
        ):
            # warm tile memset leads the gpsimd queue so the PE warmup
            # isn't gated behind the SWDGE const issues
            warm_sb = cpool.tile([128, 128 + BT], _f8, tag="warm")
            nc.gpsimd.memset(warm_sb[:], 0)

            # ---- DMA issue. Each HWDGE ring holds ~2 in-flight 128-desc
            # DMAs, so the x chunks alternate SP/ACT rings; consts ride
            # the separate SWDGE (gpsimd) queue.
            w1_sb = cpool.tile([128, (FC + 1) * MP], _f16, tag="w1")
            nc.scalar.dma_start(out=w1_sb[:], in_=w1[:])

            xch = []  # 8 chunks of 2 tiles each; one DMA per chunk
            # (fine-grained first loads backfire: each DMA completion
            # carries 2-5us of receipt jitter under load)

            def load_chunk(k, eng):
                xc = cpool.tile([128, 2 * FC * BT], _f8, tag=f"x{k}")
                eng.dma_start(
                    out=xc[:], in_=xall[:, k * 2 * FC * BT : (k + 1) * 2 * FC * BT]
                )
                xch.append(xc)

            load_chunk(0, nc.sync)
            load_chunk(1, nc.scalar)  # first on ACT ring: needed by ~16us
            xr_sb = cpool.tile([112, 4 * BT], _f8, tag="xr")
            nc.scalar.dma_start(out=xr_sb[:], in_=xr[:])
            w2_sb = cpool.tile([H1, H2], _f16, tag="w2")
            nc.scalar.dma_start(out=w2_sb[:], in_=w2[:])
            b1_sb = cpool.tile([H1, 1], _f32, tag="b1")
            nc.scalar.dma_start(out=b1_sb[:], in_=b1[:])
            b2_sb = cpool.tile([H2, 1], _f32, tag="b2")
            nc.scalar.dma_start(out=b2_sb[:], in_=b2[:])
            load_chunk(2, nc.sync)
            load_chunk(3, nc.scalar)
            load_chunk(4, nc.sync)
            load_chunk(5, nc.scalar)
            load_chunk(6, nc.sync)
            load_chunk(7, nc.scalar)

            # output staging, split by column-group parity so the DVE and
            # ACT copies never write the same tile (whole-tile WAW
            # tracking would serialize them); rows 32q+p used
            o_ev = cpool.tile([128, 4 * BT], _f32, tag="oev")
            o_od = cpool.tile([128, 4 * BT], _f32, tag="ood")

            def xslice(t, c):
                a = xch[t // 2]
                off = ((t % 2) * FC + c) * BT
                return a[:, off : off + BT]

            # PE pre-warm: full-width dummy matmuls advance the HAM clock
            # ramp while x0 streams in (short/narrow ones don't). Results
            # land in po4 and are discarded (layer-2 start=True resets it).
            warm_po = pso.tile([128, BT], _f32, tag="po", name="warm_po")
            for _ in range(N_WARM):
                nc.tensor.matmul(
                    warm_po[:],
                    warm_sb[:, :128],
                    warm_sb[:, 128:],
                    start=True,
                    stop=True,
                )

            # ---- main loop; batch s's layer-2 work is issued after batch
            # s+1's layer-1 matmuls so the PE FIFO never stalls on a relu.
            phs = {}
            hs = {}

            def l1_batch(bi):
                t0, sz = BATCHES[bi]
                for t in range(t0, t0 + sz):
                    ph = psh.tile([128, BT], _f32, tag="ph", name=f"ph{t}")
                    phs[t] = ph
                    for c in range(FC):
                        nc.tensor.matmul(
                            ph[:],
                            w1_sb[:, c * MP : (c + 1) * MP],
                            xslice(t, c),
                            start=(c == 0),
                            stop=False,
                        )
                # remainder matmuls batched: distinct PE row groups overlap
                for t in range(t0, t0 + sz):
                    q = t % 4
                    s = t // 4
                    nc.tensor.matmul(
                        phs[t][:],
                        w1_sb[32 * q : 32 * q + F_REM, FC * MP :],
                        xr_sb[32 * q : 32 * q + F_REM, s * BT : (s + 1) * BT],
                        start=False,
                        stop=True,
                        tile_position=(32 * q, 0),
                    )

            def relu_batch(bi):
                t0, sz = BATCHES[bi]
                for i, t in enumerate(range(t0, t0 + sz)):
                    h = hpool.tile([H1, BT], _f16, tag="h", name=f"h{t}")
                    hs[t] = h
                    if t % 2 == 0:
                        nc.vector.tensor_scalar(
                            h[:],
                            phs[t][0:H1, :],
                            b1_sb[:, 0:1],
                            0.0,
                            mybir.AluOpType.add,
                            mybir.AluOpType.max,
                        )
                    else:
                        nc.scalar.activation(
                            h[:], phs[t][0:H1, :], _RELU, bias=b1_sb[:, 0:1]
                        )

            pos = {}

            def l2_mm(bi):
                t0, sz = BATCHES[bi]
                # col-tiled: out partitions 32q; a batch's matmuls overlap
                # in distinct PE column groups. Fresh po tile per batch
                # (2 rotating PSUM banks) so consecutive batches' layer-2
                # chains don't serialize on whole-tile WAR tracking.
                po4 = pso.tile([128, BT], _f32, tag="po", name=f"po{bi}")
                pos[bi] = po4
                for t in range(t0, t0 + sz):
                    q = t % 4
                    nc.tensor.matmul(
                        po4[32 * q : 32 * q + H2, :],
                        w2_sb[:],
                        hs[t][:],
                        start=True,
                        stop=True,
                        tile_position=(0, 32 * q),
                    )

            def l2_copy(bi):
                t0, sz = BATCHES[bi]
                po4 = pos[bi]
                for t in range(t0, t0 + sz):
                    q = t % 4
                    s = t // 4
                    src = po4[32 * q : 32 * q + H2, :]
                    if q % 2 == 0:
                        dst = o_ev[32 * q : 32 * q + H2, s * BT : (s + 1) * BT]
                        nc.vector.tensor_scalar_add(dst, src, b2_sb[:, 0:1])
                    else:
                        dst = o_od[32 * q : 32 * q + H2, s * BT : (s + 1) * BT]
                        nc.scalar.activation(dst, src, _IDENT, bias=b2_sb[:, 0:1])
                # small per-tile stores: SBUF->HBM only engages 2 SDMA
                # engines, so big batched stores would drag a long tail
                for t in range(t0, t0 + sz):
                    q = t % 4
                    s = t // 4
                    o = o_ev if q % 2 == 0 else o_od
                    nc.sync.dma_start(
                        out=y[32 * q : 32 * q + H2, s * BT : (s + 1) * BT],
                        in_=o[32 * q : 32 * q + H2, s * BT : (s + 1) * BT],
                    )

            l1_batch(0)
            l1_batch(1)
            relu_batch(0)
            l2_mm(0)
            l1_batch(2)
            relu_batch(1)
            l2_copy(0)
            l2_mm(1)
            l1_batch(3)
            relu_batch(2)
            l2_copy(1)
            l2_mm(2)
            l1_batch(4)
            relu_batch(3)
            l2_copy(2)
            l2_mm(3)
            l1_batch(5)
            relu_batch(4)
            l2_copy(3)
            l2_mm(4)
            relu_batch(5)
            l2_copy(4)
            l2_mm(5)
            l2_copy(5)

    _split_sync_waits(nc)
    return nc


def _fold_conv_fc1(conv_w: np.ndarray, fc1_w: np.ndarray) -> np.ndarray:
    """Fold the 3x3 valid conv into fc1: W1eff[784, 100] such that
    h = x @ W1eff  ==  fc1( flatten(conv(x)) ).  Accumulated in float64."""
    F = fc1_w.astype(np.float64).T.reshape(26, 26, H1)
    W = np.zeros((28, 28, H1), np.float64)
    cw = conv_w.astype(np.float64)
    for di in range(3):
        for dj in range(3):
            W[di : di + 26, dj : dj + 26, :] += cw[di, dj] * F
    return W.reshape(784, H1)


def _q8(a: np.ndarray) -> np.ndarray:
    return np.clip(a, -15.5, 15.5).astype(ml_dtypes.float8_e3m4)


def _make_in_maps(x, conv_w, fc1_w, fc1_b, fc2_w, fc2_b):
    w1eff = _fold_conv_fc1(conv_w, fc1_w) * (1.0 / SX)  # [784, 100] f64
    W1s = np.zeros((784, MP), np.float16)
    W1s[:, :H1] = w1eff.astype(np.float16)
    w1_np = np.zeros((128, (FC + 1) * MP), np.float16)
    w1_np[:, : FC * MP] = (
        W1s[: FC * 128].reshape(FC, 128, MP).transpose(1, 0, 2).reshape(128, FC * MP)
    )
    for g in range(4):
        w1_np[32 * g : 32 * g + F_REM, FC * MP :] = W1s[FC * 128 :]
    w2_np = np.ascontiguousarray(fc2_w.T.astype(np.float16))
    b1_np = np.ascontiguousarray(fc1_b.reshape(H1, 1))
    b2_np = np.ascontiguousarray(fc2_b.reshape(H2, 1))

    in_maps = []
    for score in range(N_CORES):
        xs = x[score * B_SHARD : (score + 1) * B_SHARD].reshape(N_TILES, BT, 784)
        xq = _q8(xs * SX)  # [t, n, f] e3m4
        xall_np = np.ascontiguousarray(
            xq[:, :, : FC * 128]
            .reshape(N_TILES, BT, FC, 128)
            .transpose(3, 0, 2, 1)
            .reshape(128, N_TILES * FC * BT)
        )
        xr_np = np.zeros((112, 4 * BT), ml_dtypes.float8_e3m4)
        rem = (
            xq[:, :, FC * 128 :]
            .reshape(4, 4, BT, F_REM)  # [s, g, n, k]
            .transpose(1, 3, 0, 2)  # [g, k, s, n]
        )
        for g in range(4):
            xr_np[32 * g : 32 * g + F_REM] = rem[g].reshape(F_REM, 4 * BT)
        in_maps.append(
            {
                "xall": xall_np,
                "xr": xr_np,
                "w1": w1_np,
                "w2": w2_np,
                "b1": b1_np,
                "b2": b2_np,
            }
        )
    return in_maps


def _gather(results) -> np.ndarray:
    out = np.empty((B_TOTAL, H2), np.float32)
    for score in range(N_CORES):
        ys = results[score]["y"]  # [106, 4*BT]; rows 32q+p
        base = score * B_SHARD
        for q in range(4):
            for s in range(4):
                t = 4 * s + q
                out[base + t * BT : base + (t + 1) * BT] = ys[
                    32 * q : 32 * q + H2, s * BT : (s + 1) * BT
                ].T
    return out


def kernel_run(inputs: dict, trace: bool = False):
    """Run the kernel; returns (full output (65536,10) f32, BassKernelResults)."""
    x = np.ascontiguousarray(np.asarray(inputs["x"], dtype=np.float32))
    assert x.shape == (B_TOTAL, 784), x.shape
    in_maps = _make_in_maps(
        x,
        np.asarray(inputs["conv_w"], np.float32),
        np.asarray(inputs["fc1_w"], np.float32),
        np.asarray(inputs["fc1_b"], np.float32),
        np.asarray(inputs["fc2_w"], np.float32),
        np.asarray(inputs["fc2_b"], np.float32),
    )
    nc = _build_nc()
    res = run_bass_kernel_spmd(nc, in_maps, core_ids=list(range(N_CORES)), trace=trace)
    return _gather(res.results), res


def kernel(**inputs) -> np.ndarray:
    out, _ = kernel_run(inputs)
    return out
